# revision 32
# baseline (speedup 1.0000x reference)
"""Trainium2 kernel for nn_ChartParametrizationAD.

Reference computation (complex128):
    V = unpack(V_params)                        # (P, N) complex
    Q, R = qr([V; I_N])                         # reduced QR, LAPACK convention
    C, A = Q[:P], Q[P:]
    RHS = C^H Y ;  Lam_{k+1} = A Lam_k W + RHS  (50 steps from 0)

Key structure exploited:
  * [V; I] R^{-1} = Q  =>  A = R^{-1} (upper triangular, LAPACK signs
    included), C = V R^{-1}. Only R is needed from the QR.
  * Lam_50 = sum_{k<50} A^k RHS W^k. The spectral radius of the step map
    is ~0.35, so the series is converged far below fp32 noise by ~45
    terms. We compute S_48 = sum_{k<48} with four sum-doubling steps
    (S_{2m} = S_m + A^m S_m W^m, m = 1,2,4,8) plus a radix-3 top level
    (S_48 = S_16 + T + A^16 T W^16 with T = A^16 S_16 W^16), which needs
    no A^32/W^32 squarings. ~21 complex 512^3 GEMMs instead of 100;
    truncation error ~4e-8, far below fp32 noise.

Distribution: everything after the tiny QR is a strictly sequential
chain of 512^3 complex GEMMs (depth ~13). Measured on this fleet a 1 MB
AllReduce over 8 cores costs ~41 us while a full complex 512^3 GEMM is
~14 us, so every per-step collective scheme (2D TP per the hint,
row-sharded doubling, radix splits with per-level reduces) loses to
computing the chain on one core. All 8 cores run the same program
redundantly (SPMD, zero collectives); core 0's output is returned.

Precision: GEMM operands are float32r (fp32 storage, reduced-mantissa
multiplies, full PE rate at free-dim 512) except RHS = C^H Y in native
fp32 (RHS feeds the whole sum; the S accumulator also stays fp32).
Host computes R / A = R^{-1} / C = V A in fp64 (~1% of total flops; a
latency-bound 512-step pivot recursion unsuited to the engines).
End-to-end rel. error vs the complex128 reference: 2.5e-5;
HW exec time ~350 us.
"""

import numpy as np

N, P, NT = 512, 128, 4  # NT = N // 128 partition tiles

_CACHE = {}
_TRACE = False  # test harness sets True to collect exec_time_ns
_LAST_EXEC_NS = None


def _build_nc():
    import concourse.bacc as bacc
    import concourse.mybir as mybir
    from concourse.tile import TileContext
    from concourse.masks import make_identity

    F32 = mybir.dt.float32
    GDT = mybir.dt.float32r

    nc = bacc.Bacc("TRN2", target_bir_lowering=False)

    # ---- DRAM I/O ----
    # smalls (fp32): conj(C) planes (Cr, -Ci, +Ci) and Y planes
    cy_in = nc.dram_tensor("cy", [5 * P, N], F32, kind="ExternalInput")
    # big planes (f32r): B = A^T (r, i); Bt = A (r, i, -i); W; Wt = W^T
    def dinr(name):
        return nc.dram_tensor(name, [N, N], GDT, kind="ExternalInput")
    b_in = [dinr("b0"), dinr("b1"), dinr("b2")]
    bt_in = [dinr("bt0"), dinr("bt1"), dinr("bt2")]
    w_in = [dinr("w0"), dinr("w1")]
    wt_in = [dinr("wt0"), dinr("wt1"), dinr("wt2")]
    sr_out = nc.dram_tensor("sr", [N, N], F32, kind="ExternalOutput")
    si_out = nc.dram_tensor("si", [N, N], F32, kind="ExternalOutput")

    with TileContext(nc) as tc:
        with (
            tc.tile_pool(name="sb", bufs=1) as sb,
            tc.tile_pool(name="psum", bufs=8, space="PSUM") as psum,
        ):
            BUFS = {"s_r": 2, "s_i": 2}

            def sbtile(tag, dt=GDT):
                return sb.tile([128, NT, N], dt, tag=tag, name=tag,
                               bufs=BUFS.get(tag, 1))

            def load_plane(dram, tag):
                t = sbtile(tag)
                nc.sync.dma_start(
                    t[:, :, :], dram.rearrange("(t p) n -> p t n", p=128))
                return t

            def load_small(dram, tag):
                t = sb.tile([128, N], F32, tag=tag, name=tag, bufs=1)
                nc.sync.dma_start(t[:, :], dram[:, :])
                return t

            ident32 = sb.tile([128, 128], F32, tag="ident32",
                              name="ident32")
            make_identity(nc, ident32)
            ident = sb.tile([128, 128], GDT, tag="ident", name="ident")
            nc.vector.tensor_copy(ident[:, :], ident32[:, :])
            # HAM warmup: ~4 us of dummy matmuls while input DMAs land,
            # so the RHS/first-step matmuls run at 2.4 GHz instead of 1.2.
            ps_wu = psum.tile([128, 128], F32, tag="ps", name="ps_wu")
            for _ in range(40):
                nc.tensor.matmul(ps_wu[:, :], ident[:, :], ident[:, :],
                                 start=True, stop=True)

            t_cy = sb.tile([128, 5, N], F32, tag="cy", name="cy", bufs=1)
            nc.sync.dma_start(t_cy[:, :, :],
                              cy_in.rearrange("(j p) n -> p j n", p=128))
            t_c = [t_cy[:, j, :] for j in range(3)]
            t_y = [t_cy[:, j + 3, :] for j in range(2)]
            t_b = [load_plane(d, t) for d, t in zip(b_in, ("b_r", "b_i", "b_s"))]
            t_w = [load_plane(d, t) for d, t in zip(w_in, ("w_r", "w_i"))]
            t_bt = [load_plane(d, f"bt_{j}") for j, d in enumerate(bt_in)]
            t_wt = [load_plane(d, f"wt_{j}") for j, d in enumerate(wt_in)]

            def cgemm(lhsT, rhs, out_tag, kt=NT, add_to=None, with_neg=False,
                      with_sum=False, make_sf=False, out_dt=GDT):
                """Schoolbook complex GEMM out = lhsT^T (*) rhs.

                lhsT = (Lr, Li, nLi), rhs = (Rr, Ri).
                add_to: fp32 S planes -> out = add_to + product (fp32).
                with_neg: also produce -imag plane (for lhsT reuse).
                make_sf: also emit GDT copies (sf_r, sf_i, sf_ni) of the
                fp32 result, for the next X-hat's lhsT.
                Returns (zr, zi, nzi?) and optionally the sf triple.
                """
                Lr, Li, nLi = lhsT
                Rr, Ri = rhs

                def lsl(t, k, m):
                    return t[:, m * 128:(m + 1) * 128] if kt == 1 \
                        else t[:, k, m * 128:(m + 1) * 128]

                def rsl(t, k):
                    return t if kt == 1 else t[:, k, :]

                zr = sbtile(out_tag + "_r", out_dt)
                zi = sbtile(out_tag + "_i", out_dt)
                nzi = sbtile(out_tag + "_ni") if with_neg else None
                zs = sbtile(out_tag + "_s") if with_sum else None
                if make_sf:
                    sfr, sfi, sfs = (sbtile("sf_r"), sbtile("sf_i"),
                                     sbtile("sf_s"))
                for m in range(NT):
                    psr = psum.tile([128, N], F32, tag="ps", name="psr")
                    psi = psum.tile([128, N], F32, tag="ps", name="psi")
                    for k in range(kt):
                        nc.tensor.matmul(psr, lsl(Lr, k, m), rsl(Rr, k),
                                         start=(k == 0), stop=False)
                    for k in range(kt):
                        nc.tensor.matmul(psr, lsl(nLi, k, m), rsl(Ri, k),
                                         start=False, stop=(k == kt - 1))
                    for k in range(kt):
                        nc.tensor.matmul(psi, lsl(Lr, k, m), rsl(Ri, k),
                                         start=(k == 0), stop=False)
                    for k in range(kt):
                        nc.tensor.matmul(psi, lsl(Li, k, m), rsl(Rr, k),
                                         start=False, stop=(k == kt - 1))
                    zrm, zim = zr[:, m, :], zi[:, m, :]
                    if add_to is None:
                        nc.vector.tensor_copy(zrm, psr[:, :])
                        nc.scalar.copy(zim, psi[:, :])
                    else:
                        nc.vector.tensor_add(zrm, add_to[0][:, m, :],
                                             psr[:, :])
                        nc.vector.tensor_add(zim, add_to[1][:, m, :],
                                             psi[:, :])
                    if with_neg:
                        nc.scalar.mul(nzi[:, m, :], zim, -1.0)
                    if with_sum:
                        nc.vector.tensor_add(zs[:, m, :], zrm, zim)
                    if make_sf:
                        nc.scalar.copy(sfr[:, m, :], zrm)
                        nc.scalar.copy(sfi[:, m, :], zim)
                        nc.vector.tensor_add(sfs[:, m, :], zrm, zim)
                if make_sf:
                    return (zr, zi, nzi), (sfr, sfi, sfs)
                if with_sum:
                    return zr, zi, nzi, zs
                return zr, zi, nzi

            def kara_xh(lhsT, rhs, out_tag):
                """Karatsuba X-hat = lhsT^T (*) rhs -> (r, i, -i) GDT.
                lhsT = (Lr, Li, Ls=Lr+Li); rhs = (Rr, Ri, Rs=Rr+Ri)."""
                Lr, Li, Ls = lhsT
                Rr, Ri, Rs = rhs
                zr = sbtile(out_tag + "_r")
                zi = sbtile(out_tag + "_i")
                nzi = sbtile(out_tag + "_ni")
                for m in range(NT):
                    ps1 = psum.tile([128, N], F32, tag="ps", name="ps1")
                    ps2 = psum.tile([128, N], F32, tag="ps", name="ps2")
                    ps3 = psum.tile([128, N], F32, tag="ps", name="ps3")
                    for ps, L, Rv in ((ps1, Lr, Rr), (ps2, Li, Ri),
                                      (ps3, Ls, Rs)):
                        for k in range(NT):
                            nc.tensor.matmul(ps, L[:, k, 128*m:128*(m+1)],
                                             Rv[:, k, :], start=(k == 0),
                                             stop=(k == NT - 1))
                    zrm, zim = zr[:, m, :], zi[:, m, :]
                    nc.scalar.copy(zrm, ps1[:, :])
                    nc.vector.tensor_sub(zrm, zrm, ps2[:, :])
                    nc.scalar.copy(zim, ps3[:, :])
                    nc.vector.tensor_sub(zim, zim, ps1[:, :])
                    nc.vector.tensor_sub(zim, zim, ps2[:, :])
                    nc.scalar.mul(nzi[:, m, :], zim, -1.0)
                return zr, zi, nzi

            def transpose_mat(planes, out_tag):
                """(Mr, Mi) -> (Mtr, Mti, -Mti) via PE transposes."""
                tr = sbtile(out_tag + "_0")
                ti = sbtile(out_tag + "_1")
                nti = sbtile(out_tag + "_2")
                for src, dst, ndst in ((planes[0], tr, None),
                                       (planes[1], ti, nti)):
                    for t in range(NT):
                        pst = psum.tile([128, NT, 128], GDT, tag="ps",
                                        name="ps_t")
                        for m in range(NT):
                            nc.tensor.transpose(
                                pst[:, m, :],
                                src[:, t, m * 128:(m + 1) * 128], ident)
                        for m in range(NT):
                            nc.vector.tensor_copy(
                                dst[:, m, t * 128:(t + 1) * 128], pst[:, m, :])
                            if ndst is not None:
                                nc.scalar.mul(
                                    ndst[:, m, t * 128:(t + 1) * 128],
                                    pst[:, m, :], -1.0)
                return tr, ti, nti

            # ---- RHS = C^H Y (fp32) with fused GDT copies ----
            s, sf = cgemm((t_c[0], t_c[1], t_c[2]), (t_y[0], t_y[1]), "s",
                          kt=1, make_sf=True, out_dt=F32)
            s = (s[0], s[1])

            # ---- 4 doublings to S_16 ----
            b, bt, w, wt = t_b, t_bt, t_w, t_wt
            for i in range(4):
                xh = kara_xh(sf, b, "xh")
                s, sf = cgemm(xh, (w[0], w[1]), "s", add_to=s, make_sf=True,
                              out_dt=F32)
                s = (s[0], s[1])
                bsq = cgemm(bt, (b[0], b[1]), "b", with_sum=True)
                b = (bsq[0], bsq[1], bsq[3])            # B <- B^2 (r, i, sum)
                w = cgemm(wt, (w[0], w[1]), "w")
                if i < 3:
                    bt = transpose_mat(b, "bt")
                    wt = transpose_mat(w, "wt")

            # ---- radix-3 top: T = A^16 S_16 W^16 ----
            # T lands in the sf slots (GDT triple) AND s <- S_16 + T.
            xh = kara_xh(sf, b, "xh")
            # T = A^16 S_16 W^16 into its own GDT planes (with sum for the
            # next X-hat's Karatsuba lhsT), then S_32 = S_16 + T.
            t16 = cgemm(xh, (w[0], w[1]), "t16", with_sum=True)
            # S_32 = S_16 + T  (DVE adds, SBUF 2x)
            s32r, s32i = sbtile("s_r", F32), sbtile("s_i", F32)
            for m in range(NT):
                nc.vector.tensor_add(s32r[:, m, :], s[0][:, m, :],
                                     t16[0][:, m, :])
                nc.vector.tensor_add(s32i[:, m, :], s[1][:, m, :],
                                     t16[1][:, m, :])
            # S_48 = S_32 + A^16 T W^16
            xh = kara_xh((t16[0], t16[1], t16[3]), b, "xh")
            s = cgemm(xh, (w[0], w[1]), "s", add_to=(s32r, s32i), out_dt=F32)

            # ---- store ----
            sr_v = sr_out.rearrange("(t p) n -> p t n", p=128)
            si_v = si_out.rearrange("(t p) n -> p t n", p=128)
            for m in range(NT):
                nc.sync.dma_start(sr_v[:, m, :], s[0][:, m, :])
                nc.sync.dma_start(si_v[:, m, :], s[1][:, m, :])

    nc.compile()
    return nc


def _get_nc():
    if "nc" not in _CACHE:
        _CACHE["nc"] = _build_nc()
    return _CACHE["nc"]


def kernel(V_params, W_real, W_imag, Y_real, Y_imag):
    global _LAST_EXEC_NS
    from concourse.bass_utils import run_bass_kernel_spmd

    # ---- host: deparametrize in fp64 (QR of [V; I], LAPACK convention) ----
    Vp = np.asarray(V_params, dtype=np.float64)
    V = Vp[:N * P].reshape(P, N) + 1j * Vp[N * P:].reshape(P, N)
    stacked = np.concatenate([V, np.eye(N, dtype=np.complex128)], axis=0)
    _, R = np.linalg.qr(stacked)          # reduced; R carries the signs
    A = np.linalg.inv(R)                  # = Q[P:], upper triangular
    C = V @ A                             # = Q[:P]

    f32 = np.float32

    def c(x):
        return np.ascontiguousarray(x, dtype=f32)

    Wr = np.asarray(W_real, np.float64)
    Wi = np.asarray(W_imag, np.float64)
    AT = A.T
    in_map = {
        "cy": c(np.concatenate([
            C.real, -C.imag, C.imag,
            np.asarray(Y_real, np.float64), np.asarray(Y_imag, np.float64),
        ], axis=0)),
        "b0": c(AT.real), "b1": c(AT.imag), "b2": c(AT.real + AT.imag),
        "bt0": c(A.real), "bt1": c(A.imag), "bt2": c(-A.imag),
        "w0": c(Wr), "w1": c(Wi),
        "wt0": c(Wr.T), "wt1": c(Wi.T), "wt2": c(-Wi.T),
    }

    nc = _get_nc()
    res = None
    for attempt in range(3):
        try:
            res = run_bass_kernel_spmd(nc, [in_map] * 8,
                                       core_ids=list(range(8)), trace=_TRACE)
            break
        except Exception:
            if attempt == 2:
                raise
    _LAST_EXEC_NS = res.exec_time_ns
    _CACHE["last_res"] = res
    out = res.results[0]
    lam = out["sr"].astype(np.float64) + 1j * out["si"].astype(np.float64)
    return lam


# revision 33
# speedup vs baseline: 1.0088x; 1.0088x over previous
"""Trainium2 kernel for nn_ChartParametrizationAD.

Reference computation (complex128):
    V = unpack(V_params)                        # (P, N) complex
    Q, R = qr([V; I_N])                         # reduced QR, LAPACK convention
    C, A = Q[:P], Q[P:]
    RHS = C^H Y ;  Lam_{k+1} = A Lam_k W + RHS  (50 steps from 0)

Key structure exploited:
  * [V; I] R^{-1} = Q  =>  A = R^{-1} (upper triangular, LAPACK signs
    included), C = V R^{-1}. Only R is needed from the QR.
  * Lam_50 = sum_{k<50} A^k RHS W^k. The spectral radius of the step map
    is ~0.35, so the series is converged far below fp32 noise by ~45
    terms. We compute S_48 = sum_{k<48} with four sum-doubling steps
    (S_{2m} = S_m + A^m S_m W^m, m = 1,2,4,8) plus a radix-3 top level
    (S_48 = S_16 + T + A^16 T W^16 with T = A^16 S_16 W^16), which needs
    no A^32/W^32 squarings. ~21 complex 512^3 GEMMs instead of 100;
    truncation error ~4e-8, far below fp32 noise.

Distribution: everything after the tiny QR is a strictly sequential
chain of 512^3 complex GEMMs (depth ~13). Measured on this fleet a 1 MB
AllReduce over 8 cores costs ~41 us while a full complex 512^3 GEMM is
~14 us, so every per-step collective scheme (2D TP per the hint,
row-sharded doubling, radix splits with per-level reduces) loses to
computing the chain on one core. All 8 cores run the same program
redundantly (SPMD, zero collectives); core 0's output is returned.

Precision: GEMM operands are float32r (fp32 storage, reduced-mantissa
multiplies, full PE rate at free-dim 512) except RHS = C^H Y in native
fp32 (RHS feeds the whole sum; the S accumulator also stays fp32).
Host computes R / A = R^{-1} / C = V A in fp64 (~1% of total flops; a
latency-bound 512-step pivot recursion unsuited to the engines).
End-to-end rel. error vs the complex128 reference: 2.5e-5;
HW exec time ~348 us.
"""

import numpy as np

N, P, NT = 512, 128, 4  # NT = N // 128 partition tiles

_CACHE = {}
_TRACE = False  # test harness sets True to collect exec_time_ns
_LAST_EXEC_NS = None


def _build_nc():
    import concourse.bacc as bacc
    import concourse.mybir as mybir
    from concourse.tile import TileContext
    from concourse.masks import make_identity

    F32 = mybir.dt.float32
    GDT = mybir.dt.float32r

    nc = bacc.Bacc("TRN2", target_bir_lowering=False)

    # ---- DRAM I/O ----
    # smalls (fp32): conj(C) planes (Cr, -Ci, +Ci) and Y planes
    cy_in = nc.dram_tensor("cy", [5 * P, N], F32, kind="ExternalInput")
    # big planes (f32r): B = A^T (r, i); Bt = A (r, i, -i); W; Wt = W^T
    def dinr(name):
        return nc.dram_tensor(name, [N, N], GDT, kind="ExternalInput")
    b_in = [dinr("b0"), dinr("b1"), dinr("b2")]
    bt_in = [dinr("bt0"), dinr("bt1"), dinr("bt2")]
    w_in = [dinr("w0"), dinr("w1")]
    wt_in = [dinr("wt0"), dinr("wt1"), dinr("wt2")]
    sr_out = nc.dram_tensor("sr", [N, N], F32, kind="ExternalOutput")
    si_out = nc.dram_tensor("si", [N, N], F32, kind="ExternalOutput")

    with TileContext(nc) as tc:
        with (
            tc.tile_pool(name="sb", bufs=1) as sb,
            tc.tile_pool(name="psum", bufs=8, space="PSUM") as psum,
        ):
            BUFS = {"s_r": 2, "s_i": 2}

            def sbtile(tag, dt=GDT):
                return sb.tile([128, NT, N], dt, tag=tag, name=tag,
                               bufs=BUFS.get(tag, 1))

            def load_plane(dram, tag):
                t = sbtile(tag)
                nc.sync.dma_start(
                    t[:, :, :], dram.rearrange("(t p) n -> p t n", p=128))
                return t

            def load_small(dram, tag):
                t = sb.tile([128, N], F32, tag=tag, name=tag, bufs=1)
                nc.sync.dma_start(t[:, :], dram[:, :])
                return t

            ident32 = sb.tile([128, 128], F32, tag="ident32",
                              name="ident32")
            make_identity(nc, ident32)
            ident = sb.tile([128, 128], GDT, tag="ident", name="ident")
            nc.vector.tensor_copy(ident[:, :], ident32[:, :])

            t_cy = sb.tile([128, 5, N], F32, tag="cy", name="cy", bufs=1)
            nc.sync.dma_start(t_cy[:, :, :],
                              cy_in.rearrange("(j p) n -> p j n", p=128))
            t_c = [t_cy[:, j, :] for j in range(3)]
            t_y = [t_cy[:, j + 3, :] for j in range(2)]
            t_b = [load_plane(d, t) for d, t in zip(b_in, ("b_r", "b_i", "b_s"))]
            t_w = [load_plane(d, t) for d, t in zip(w_in, ("w_r", "w_i"))]
            t_bt = [load_plane(d, f"bt_{j}") for j, d in enumerate(bt_in)]
            t_wt = [load_plane(d, f"wt_{j}") for j, d in enumerate(wt_in)]

            def cgemm(lhsT, rhs, out_tag, kt=NT, add_to=None, with_neg=False,
                      with_sum=False, make_sf=False, out_dt=GDT):
                """Schoolbook complex GEMM out = lhsT^T (*) rhs.

                lhsT = (Lr, Li, nLi), rhs = (Rr, Ri).
                add_to: fp32 S planes -> out = add_to + product (fp32).
                with_neg: also produce -imag plane (for lhsT reuse).
                make_sf: also emit GDT copies (sf_r, sf_i, sf_ni) of the
                fp32 result, for the next X-hat's lhsT.
                Returns (zr, zi, nzi?) and optionally the sf triple.
                """
                Lr, Li, nLi = lhsT
                Rr, Ri = rhs

                def lsl(t, k, m):
                    return t[:, m * 128:(m + 1) * 128] if kt == 1 \
                        else t[:, k, m * 128:(m + 1) * 128]

                def rsl(t, k):
                    return t if kt == 1 else t[:, k, :]

                zr = sbtile(out_tag + "_r", out_dt)
                zi = sbtile(out_tag + "_i", out_dt)
                nzi = sbtile(out_tag + "_ni") if with_neg else None
                zs = sbtile(out_tag + "_s") if with_sum else None
                if make_sf:
                    sfr, sfi, sfs = (sbtile("sf_r"), sbtile("sf_i"),
                                     sbtile("sf_s"))
                for m in range(NT):
                    psr = psum.tile([128, N], F32, tag="ps", name="psr")
                    psi = psum.tile([128, N], F32, tag="ps", name="psi")
                    for k in range(kt):
                        nc.tensor.matmul(psr, lsl(Lr, k, m), rsl(Rr, k),
                                         start=(k == 0), stop=False)
                    for k in range(kt):
                        nc.tensor.matmul(psr, lsl(nLi, k, m), rsl(Ri, k),
                                         start=False, stop=(k == kt - 1))
                    for k in range(kt):
                        nc.tensor.matmul(psi, lsl(Lr, k, m), rsl(Ri, k),
                                         start=(k == 0), stop=False)
                    for k in range(kt):
                        nc.tensor.matmul(psi, lsl(Li, k, m), rsl(Rr, k),
                                         start=False, stop=(k == kt - 1))
                    zrm, zim = zr[:, m, :], zi[:, m, :]
                    if add_to is None:
                        nc.vector.tensor_copy(zrm, psr[:, :])
                        nc.scalar.copy(zim, psi[:, :])
                    else:
                        nc.vector.tensor_add(zrm, add_to[0][:, m, :],
                                             psr[:, :])
                        nc.vector.tensor_add(zim, add_to[1][:, m, :],
                                             psi[:, :])
                    if with_neg:
                        nc.scalar.mul(nzi[:, m, :], zim, -1.0)
                    if with_sum:
                        nc.vector.tensor_add(zs[:, m, :], zrm, zim)
                    if make_sf:
                        nc.scalar.copy(sfr[:, m, :], zrm)
                        nc.scalar.copy(sfi[:, m, :], zim)
                        nc.vector.tensor_add(sfs[:, m, :], zrm, zim)
                if make_sf:
                    return (zr, zi, nzi), (sfr, sfi, sfs)
                if with_sum:
                    return zr, zi, nzi, zs
                return zr, zi, nzi

            def kara_xh(lhsT, rhs, out_tag):
                """Karatsuba X-hat = lhsT^T (*) rhs -> (r, i, -i) GDT.
                lhsT = (Lr, Li, Ls=Lr+Li); rhs = (Rr, Ri, Rs=Rr+Ri)."""
                Lr, Li, Ls = lhsT
                Rr, Ri, Rs = rhs
                zr = sbtile(out_tag + "_r")
                zi = sbtile(out_tag + "_i")
                nzi = sbtile(out_tag + "_ni")
                for m in range(NT):
                    ps1 = psum.tile([128, N], F32, tag="ps", name="ps1")
                    ps2 = psum.tile([128, N], F32, tag="ps", name="ps2")
                    ps3 = psum.tile([128, N], F32, tag="ps", name="ps3")
                    for ps, L, Rv in ((ps1, Lr, Rr), (ps2, Li, Ri),
                                      (ps3, Ls, Rs)):
                        for k in range(NT):
                            nc.tensor.matmul(ps, L[:, k, 128*m:128*(m+1)],
                                             Rv[:, k, :], start=(k == 0),
                                             stop=(k == NT - 1))
                    zrm, zim = zr[:, m, :], zi[:, m, :]
                    nc.scalar.copy(zrm, ps1[:, :])
                    nc.vector.tensor_sub(zrm, zrm, ps2[:, :])
                    nc.scalar.copy(zim, ps3[:, :])
                    nc.vector.tensor_sub(zim, zim, ps1[:, :])
                    nc.vector.tensor_sub(zim, zim, ps2[:, :])
                    nc.scalar.mul(nzi[:, m, :], zim, -1.0)
                return zr, zi, nzi

            def transpose_mat(planes, out_tag):
                """(Mr, Mi) -> (Mtr, Mti, -Mti) via PE transposes."""
                tr = sbtile(out_tag + "_0")
                ti = sbtile(out_tag + "_1")
                nti = sbtile(out_tag + "_2")
                for src, dst, ndst in ((planes[0], tr, None),
                                       (planes[1], ti, nti)):
                    for t in range(NT):
                        pst = psum.tile([128, NT, 128], GDT, tag="ps",
                                        name="ps_t")
                        for m in range(NT):
                            nc.tensor.transpose(
                                pst[:, m, :],
                                src[:, t, m * 128:(m + 1) * 128], ident)
                        for m in range(NT):
                            nc.vector.tensor_copy(
                                dst[:, m, t * 128:(t + 1) * 128], pst[:, m, :])
                            if ndst is not None:
                                nc.scalar.mul(
                                    ndst[:, m, t * 128:(t + 1) * 128],
                                    pst[:, m, :], -1.0)
                return tr, ti, nti

            # ---- RHS = C^H Y (fp32) with fused GDT copies ----
            s, sf = cgemm((t_c[0], t_c[1], t_c[2]), (t_y[0], t_y[1]), "s",
                          kt=1, make_sf=True, out_dt=F32)
            s = (s[0], s[1])

            # ---- 4 doublings to S_16 ----
            b, bt, w, wt = t_b, t_bt, t_w, t_wt
            for i in range(4):
                xh = kara_xh(sf, b, "xh")
                s, sf = cgemm(xh, (w[0], w[1]), "s", add_to=s, make_sf=True,
                              out_dt=F32)
                s = (s[0], s[1])
                bsq = cgemm(bt, (b[0], b[1]), "b", with_sum=True)
                b = (bsq[0], bsq[1], bsq[3])            # B <- B^2 (r, i, sum)
                w = cgemm(wt, (w[0], w[1]), "w")
                if i < 3:
                    bt = transpose_mat(b, "bt")
                    wt = transpose_mat(w, "wt")

            # ---- radix-3 top: T = A^16 S_16 W^16 ----
            # T lands in the sf slots (GDT triple) AND s <- S_16 + T.
            xh = kara_xh(sf, b, "xh")
            # T = A^16 S_16 W^16 into its own GDT planes (with sum for the
            # next X-hat's Karatsuba lhsT), then S_32 = S_16 + T.
            t16 = cgemm(xh, (w[0], w[1]), "t16", with_sum=True)
            # S_32 = S_16 + T  (DVE adds, SBUF 2x)
            s32r, s32i = sbtile("s_r", F32), sbtile("s_i", F32)
            for m in range(NT):
                nc.vector.tensor_add(s32r[:, m, :], s[0][:, m, :],
                                     t16[0][:, m, :])
                nc.vector.tensor_add(s32i[:, m, :], s[1][:, m, :],
                                     t16[1][:, m, :])
            # S_48 = S_32 + A^16 T W^16
            xh = kara_xh((t16[0], t16[1], t16[3]), b, "xh")
            s = cgemm(xh, (w[0], w[1]), "s", add_to=(s32r, s32i), out_dt=F32)

            # ---- store ----
            sr_v = sr_out.rearrange("(t p) n -> p t n", p=128)
            si_v = si_out.rearrange("(t p) n -> p t n", p=128)
            for m in range(NT):
                nc.sync.dma_start(sr_v[:, m, :], s[0][:, m, :])
                nc.sync.dma_start(si_v[:, m, :], s[1][:, m, :])

    nc.compile()
    return nc


def _get_nc():
    if "nc" not in _CACHE:
        _CACHE["nc"] = _build_nc()
    return _CACHE["nc"]


def kernel(V_params, W_real, W_imag, Y_real, Y_imag):
    global _LAST_EXEC_NS
    from concourse.bass_utils import run_bass_kernel_spmd

    # ---- host: deparametrize in fp64 (QR of [V; I], LAPACK convention) ----
    Vp = np.asarray(V_params, dtype=np.float64)
    V = Vp[:N * P].reshape(P, N) + 1j * Vp[N * P:].reshape(P, N)
    stacked = np.concatenate([V, np.eye(N, dtype=np.complex128)], axis=0)
    _, R = np.linalg.qr(stacked)          # reduced; R carries the signs
    A = np.linalg.inv(R)                  # = Q[P:], upper triangular
    C = V @ A                             # = Q[:P]

    f32 = np.float32

    def c(x):
        return np.ascontiguousarray(x, dtype=f32)

    Wr = np.asarray(W_real, np.float64)
    Wi = np.asarray(W_imag, np.float64)
    AT = A.T
    in_map = {
        "cy": c(np.concatenate([
            C.real, -C.imag, C.imag,
            np.asarray(Y_real, np.float64), np.asarray(Y_imag, np.float64),
        ], axis=0)),
        "b0": c(AT.real), "b1": c(AT.imag), "b2": c(AT.real + AT.imag),
        "bt0": c(A.real), "bt1": c(A.imag), "bt2": c(-A.imag),
        "w0": c(Wr), "w1": c(Wi),
        "wt0": c(Wr.T), "wt1": c(Wi.T), "wt2": c(-Wi.T),
    }

    nc = _get_nc()
    res = None
    for attempt in range(3):
        try:
            res = run_bass_kernel_spmd(nc, [in_map] * 8,
                                       core_ids=list(range(8)), trace=_TRACE)
            break
        except Exception:
            if attempt == 2:
                raise
    _LAST_EXEC_NS = res.exec_time_ns
    _CACHE["last_res"] = res
    out = res.results[0]
    lam = out["sr"].astype(np.float64) + 1j * out["si"].astype(np.float64)
    return lam
